# revision 2
# baseline (speedup 1.0000x reference)
"""Trainium2 Bass kernel for nn_ModelNew_3556232921881 (dense_mlp).

Computes, for x[4096,4096], weight[4096,4096], bias[4096]:
    y = x @ weight.T + bias
    per-256-column-block mean subtraction (divided by out_features)
    tanh-approx GELU with clamped tanh

Sharding: 2 batch shards x 4 out-feature shards across 8 NeuronCores.
Per core: M=2048, N=1024, K=4096 GEMM (fp32r full-rate matmul) with a
fused epilogue (bias add -> block reduce -> Gelu_apprx_tanh with the
negated block mean as per-partition activation bias).

Host side pre-rounds x/weight to fp32r (fp32 with 11-bit mantissa, RNE)
and swizzles them into the exact SBUF layouts so the device does zero
transposes or dtype conversions.
"""

import numpy as np
from contextlib import ExitStack

B, IN_F, OUT_F = 4096, 4096, 4096
P_B, P_O = 2, 4          # batch shards x out-feature shards
MB = B // P_B            # 2048 rows per core
NB = OUT_F // P_O        # 1024 out cols per core
K = IN_F
P = 128
M_TILES = MB // P        # 16
KO = K // P              # 32
N_TILES = NB // 512      # 2
N_CORES = 8

_STATE: dict = {}


def _round_fp32r(a: np.ndarray) -> np.ndarray:
    """Round fp32 to fp32r (11-bit mantissa, RNE) — matches walrus
    fp32_to_fp32r."""
    b = np.ascontiguousarray(a, dtype=np.float32).view(np.uint32)
    lsb = (b >> np.uint32(12)) & np.uint32(1)
    return ((b + np.uint32(0x7FF) + lsb) & np.uint32(0xFFFFF000)).view(np.float32)


def _build_bass():
    import concourse.bass as bass  # noqa: F401
    import concourse.tile as tile
    from concourse import bacc, mybir

    f32 = mybir.dt.float32
    f32r = mybir.dt.float32r
    AF = mybir.ActivationFunctionType

    nc = bacc.Bacc("TRN2", target_bir_lowering=False, debug=False)

    # element [p, m, ko, b] = xr[m*128+b, ko*128+p]  (per-core batch shard)
    xs_d = nc.dram_tensor("xs", [P, M_TILES, KO, P], f32r, kind="ExternalInput")
    # element [p, ko, n] = w[n, ko*128+p]            (per-core outf shard)
    ws_d = nc.dram_tensor("ws", [P, KO, NB], f32r, kind="ExternalInput")
    bb_d = nc.dram_tensor("bb", [P, NB], f32, kind="ExternalInput")
    out_d = nc.dram_tensor("out", [MB, NB], f32, kind="ExternalOutput")

    with tile.TileContext(nc) as tc:
        with ExitStack() as ctx:
            wpool = ctx.enter_context(tc.tile_pool(name="w", bufs=1))
            xpool = ctx.enter_context(tc.tile_pool(name="x", bufs=2))
            ypool = ctx.enter_context(tc.tile_pool(name="y", bufs=3))
            gpool = ctx.enter_context(tc.tile_pool(name="g", bufs=3))
            spool = ctx.enter_context(tc.tile_pool(name="s", bufs=3))
            psum = ctx.enter_context(tc.tile_pool(name="ps", bufs=4, space="PSUM"))

            # first x chunk early so PE can start while W streams in
            xt0 = xpool.tile([P, KO, P], f32r, name="xt")
            nc.sync.dma_start(xt0[:], xs_d.ap()[:, 0])

            bb_t = wpool.tile([P, NB], f32, name="bb")
            nc.sync.dma_start(bb_t[:], bb_d.ap())

            wts = []
            for ko in range(KO):
                wt = wpool.tile([P, NB], f32r, name=f"wt{ko}")
                nc.sync.dma_start(wt[:], ws_d.ap()[:, ko])
                wts.append(wt)

            xts = {0: xt0}
            for m in range(M_TILES):
                if m + 1 < M_TILES:
                    xt_next = xpool.tile([P, KO, P], f32r, name="xt")
                    nc.sync.dma_start(xt_next[:], xs_d.ap()[:, m + 1])
                    xts[m + 1] = xt_next
                xt = xts.pop(m)

                ps = [psum.tile([P, 512], f32, name="ps") for _ in range(N_TILES)]
                for ko in range(KO):
                    for n in range(N_TILES):
                        nc.tensor.matmul(
                            ps[n][:],
                            xt[:, ko],
                            wts[ko][:, n * 512 : (n + 1) * 512],
                            start=(ko == 0),
                            stop=(ko == KO - 1),
                        )

                for n in range(N_TILES):
                    nsl = slice(n * 512, (n + 1) * 512)
                    y1 = ypool.tile([P, 512], f32, name="y1")
                    nc.vector.tensor_add(y1[:], ps[n][:], bb_t[:, nsl])
                    s = spool.tile([P, 2], f32, name="s")
                    nc.vector.reduce_sum(
                        s[:],
                        y1[:].rearrange("p (b f) -> p b f", f=256),
                        axis=mybir.AxisListType.X,
                    )
                    nm = spool.tile([P, 2], f32, name="nm")
                    nc.vector.tensor_scalar_mul(nm[:], s[:], -1.0 / OUT_F)
                    g = gpool.tile([P, 512], f32, name="g")
                    for h in range(2):
                        nc.scalar.activation(
                            g[:, h * 256 : (h + 1) * 256],
                            y1[:, h * 256 : (h + 1) * 256],
                            AF.Gelu_apprx_tanh,
                            bias=nm[:, h : h + 1],
                        )
                    nc.sync.dma_start(
                        out_d.ap()[m * P : (m + 1) * P, nsl], g[:]
                    )

    nc.compile()
    return nc


def _get_runner():
    """Build (once) a jitted 8-core shard_map runner for the compiled Bass
    module. Returns (fn, n_params, out_shape) where fn takes concatenated
    per-core inputs [xs, ws, bb, out_zeros] and returns the concat output."""
    if "runner" in _STATE:
        return _STATE["runner"]

    import jax
    from jax.experimental.shard_map import shard_map
    from jax.sharding import Mesh, PartitionSpec
    from concourse import mybir
    from concourse.bass2jax import (
        _bass_exec_p,
        install_neuronx_cc_hook,
        partition_id_tensor,
    )

    nc = _build_bass()
    install_neuronx_cc_hook()

    partition_name = nc.partition_id_tensor.name if nc.partition_id_tensor else None
    in_names = []
    out_names = []
    out_avals = []
    for alloc in nc.m.functions[0].allocations:
        if not isinstance(alloc, mybir.MemoryLocationSet):
            continue
        name = alloc.memorylocations[0].name
        if alloc.kind == "ExternalInput":
            if name != partition_name:
                in_names.append(name)
        elif alloc.kind == "ExternalOutput":
            out_names.append(name)
            out_avals.append(
                jax.core.ShapedArray(tuple(alloc.tensor_shape), mybir.dt.np(alloc.dtype))
            )
    n_params = len(in_names)
    in_names = in_names + out_names
    if partition_name is not None:
        in_names.append(partition_name)

    def _body(*args):
        operands = list(args)
        if partition_name is not None:
            operands.append(partition_id_tensor())
        outs = _bass_exec_p.bind(
            *operands,
            out_avals=tuple(out_avals),
            in_names=tuple(in_names),
            out_names=tuple(out_names),
            lowering_input_output_aliases=(),
            sim_require_finite=True,
            sim_require_nnan=True,
            nc=nc,
        )
        return tuple(outs)

    devices = jax.devices()[:N_CORES]
    mesh = Mesh(np.asarray(devices), ("core",))
    n_outs = len(out_names)
    fn = jax.jit(
        shard_map(
            _body,
            mesh=mesh,
            in_specs=(PartitionSpec("core"),) * (n_params + n_outs),
            out_specs=(PartitionSpec("core"),) * n_outs,
            check_rep=False,
        ),
        keep_unused=True,
    )
    _STATE["runner"] = (fn, tuple(in_names[:n_params]), out_avals)
    return _STATE["runner"]


def _prepare_inputs(x, weight, bias):
    """Round + shard + swizzle. Returns dict name -> concatenated (8*dim0)
    numpy array matching the runner's input order."""
    xr = _round_fp32r(x)
    wr = _round_fp32r(weight)
    bias = np.ascontiguousarray(bias, dtype=np.float32)

    xs_l, ws_l, bb_l = [], [], []
    for c in range(N_CORES):
        bi, oj = divmod(c, P_O)
        xc = xr[bi * MB : (bi + 1) * MB, :]
        # [p, m, ko, b] = xc[m*128+b, ko*128+p]
        xs = np.ascontiguousarray(
            xc.reshape(M_TILES, P, KO, P).transpose(3, 0, 2, 1)
        )
        wc = wr[oj * NB : (oj + 1) * NB, :]
        # [p, ko, n] = wc[n, ko*128+p]
        ws = np.ascontiguousarray(wc.reshape(NB, KO, P).transpose(2, 1, 0))
        bb = np.ascontiguousarray(
            np.broadcast_to(bias[oj * NB : (oj + 1) * NB], (P, NB))
        )
        xs_l.append(xs)
        ws_l.append(ws)
        bb_l.append(bb)
    return {
        "xs": np.concatenate(xs_l, axis=0),
        "ws": np.concatenate(ws_l, axis=0),
        "bb": np.concatenate(bb_l, axis=0),
    }


def _assemble(out_concat: np.ndarray) -> np.ndarray:
    """[8*2048, 1024] per-core stack -> full [4096, 4096]."""
    y = np.empty((B, OUT_F), np.float32)
    per = out_concat.reshape(N_CORES, MB, NB)
    for c in range(N_CORES):
        bi, oj = divmod(c, P_O)
        y[bi * MB : (bi + 1) * MB, oj * NB : (oj + 1) * NB] = per[c]
    return y


def kernel(x: np.ndarray, weight: np.ndarray, bias: np.ndarray) -> np.ndarray:
    fn, param_names, out_avals = _get_runner()
    ins = _prepare_inputs(np.asarray(x), np.asarray(weight), np.asarray(bias))
    args = [ins[n] for n in param_names]
    zeros = [
        np.zeros((N_CORES * a.shape[0], *a.shape[1:]), a.dtype) for a in out_avals
    ]
    outs = fn(*args, *zeros)
    return _assemble(np.asarray(outs[0]))


# revision 6
# speedup vs baseline: 10.1180x; 10.1180x over previous
"""Trainium2 Bass kernel for nn_ModelNew_3556232921881 (dense_mlp).

Computes, for x[4096,4096], weight[4096,4096], bias[4096]:
    y = x @ weight.T + bias
    per-256-column-block mean subtraction (divided by out_features)
    tanh-approx GELU with clamped tanh

Sharding: 2 batch shards x 4 out-feature shards across 8 NeuronCores.
Per core: M=2048, N=1024, K=4096 GEMM (fp32r full-rate matmul) with a
fused epilogue (bias add -> block reduce -> Gelu_apprx_tanh with the
negated block mean as per-partition activation bias).

Host side pre-rounds x/weight to fp32r (fp32 with 11-bit mantissa, RNE)
and swizzles them into the exact SBUF layouts so the device does zero
transposes or dtype conversions. The W shard (16MB) is SBUF-resident;
x streams per 128-row tile. The first 4 m-tiles run k-synchronously
with the W DMA stream so the PE never waits for the W preload.
"""

import numpy as np
from contextlib import ExitStack

B, IN_F, OUT_F = 4096, 4096, 4096
P_B, P_O = 2, 4          # batch shards x out-feature shards
MB = B // P_B            # 2048 rows per core
NB = OUT_F // P_O        # 1024 out cols per core
K = IN_F
P = 128
M_TILES = MB // P        # 16
KO = K // P              # 32
N_TILES = NB // 512      # 2
N_CORES = 8
WARM_G = 4               # m-tiles processed k-synchronously with W stream

_STATE: dict = {}


def _round_fp32r(a: np.ndarray) -> np.ndarray:
    """Round fp32 to fp32r (11-bit mantissa, RNE) — matches walrus
    fp32_to_fp32r."""
    b = np.ascontiguousarray(a, dtype=np.float32).view(np.uint32)
    lsb = (b >> np.uint32(12)) & np.uint32(1)
    return ((b + np.uint32(0x7FF) + lsb) & np.uint32(0xFFFFF000)).view(np.float32)


def _build_bass(loop_reps=None, warm_group=WARM_G):
    import concourse.bass as bass  # noqa: F401
    import concourse.tile as tile
    from concourse import bacc, mybir

    f32 = mybir.dt.float32
    f32r = mybir.dt.float32r
    AF = mybir.ActivationFunctionType

    nc = bacc.Bacc("TRN2", target_bir_lowering=False, debug=False)

    # element [p, m, ko, b] = xr[m*128+b, ko*128+p]  (per-core batch shard)
    xs_d = nc.dram_tensor("xs", [P, M_TILES, KO, P], f32r, kind="ExternalInput")
    # element [p, ko, n] = w[n, ko*128+p]            (per-core outf shard)
    ws_d = nc.dram_tensor("ws", [P, KO, NB], f32r, kind="ExternalInput")
    bb_d = nc.dram_tensor("bb", [P, NB], f32, kind="ExternalInput")
    out_d = nc.dram_tensor("out", [MB, NB], f32, kind="ExternalOutput")

    with tile.TileContext(nc) as tc:
        with ExitStack() as ctx:
            wpool = ctx.enter_context(tc.tile_pool(name="w", bufs=1))
            xpool = ctx.enter_context(tc.tile_pool(name="x", bufs=max(warm_group, 2) + 0 if warm_group else 2))
            ypool = ctx.enter_context(tc.tile_pool(name="y", bufs=2))
            gpool = ctx.enter_context(tc.tile_pool(name="g", bufs=2))
            spool = ctx.enter_context(tc.tile_pool(name="s", bufs=3))
            psum = ctx.enter_context(tc.tile_pool(name="ps", bufs=8, space="PSUM"))

            def epilogue(m, n, ps_t, bb_t):
                nsl = slice(n * 512, (n + 1) * 512)
                y1 = ypool.tile([P, 512], f32, name="y1")
                nc.vector.tensor_add(y1[:], ps_t[:], bb_t[:, nsl])
                s = spool.tile([P, 2], f32, name="s")
                nc.vector.reduce_sum(
                    s[:],
                    y1[:].rearrange("p (b f) -> p b f", f=256),
                    axis=mybir.AxisListType.X,
                )
                nm = spool.tile([P, 2], f32, name="nm")
                nc.vector.tensor_scalar_mul(nm[:], s[:], -1.0 / OUT_F)
                g = gpool.tile([P, 512], f32, name="g")
                for h in range(2):
                    nc.scalar.activation(
                        g[:, h * 256 : (h + 1) * 256],
                        y1[:, h * 256 : (h + 1) * 256],
                        AF.Gelu_apprx_tanh,
                        bias=nm[:, h : h + 1],
                    )
                nc.sync.dma_start(out_d.ap()[m * P : (m + 1) * P, nsl], g[:])

            def body():
                G = warm_group
                # -- phase 0: early DMAs, x chunks for the warm group
                xts = {}
                for m in range(min(G, 2)):
                    xts[m] = xpool.tile([P, KO, P], f32r, name="xt")
                    nc.sync.dma_start(xts[m][:], xs_d.ap()[:, m])

                bb_t = wpool.tile([P, NB], f32, name="bb")
                nc.sync.dma_start(bb_t[:], bb_d.ap())

                wts = []
                for ko in range(KO):
                    wt = wpool.tile([P, NB], f32r, name=f"wt{ko}")
                    nc.sync.dma_start(wt[:], ws_d.ap()[:, ko])
                    wts.append(wt)
                    if ko < G - 2:  # remaining warm-group x chunks, interleaved
                        m = ko + 2
                        xts[m] = xpool.tile([P, KO, P], f32r, name="xt")
                        nc.sync.dma_start(xts[m][:], xs_d.ap()[:, m])

                # -- phase 1: warm group, k-synchronous with W arrival
                if G:
                    ps1 = {
                        (m, n): psum.tile([P, 512], f32, name="ps")
                        for m in range(G)
                        for n in range(N_TILES)
                    }
                    for ko in range(KO):
                        for m in range(G):
                            for n in range(N_TILES):
                                nc.tensor.matmul(
                                    ps1[m, n][:],
                                    xts[m][:, ko],
                                    wts[ko][:, n * 512 : (n + 1) * 512],
                                    start=(ko == 0),
                                    stop=(ko == KO - 1),
                                )
                    # prefetch next x chunk (reuses slot freed at phase-1 end)
                    if G < M_TILES:
                        xts[G] = xpool.tile([P, KO, P], f32r, name="xt")
                        nc.sync.dma_start(xts[G][:], xs_d.ap()[:, G])
                    for m in range(G):
                        del xts[m]
                        for n in range(N_TILES):
                            epilogue(m, n, ps1[m, n], bb_t)

                # -- phase 2: remaining m-tiles, k-inner per tile
                for m in range(G, M_TILES):
                    if m + 1 < M_TILES and (m + 1) not in xts:
                        xts[m + 1] = xpool.tile([P, KO, P], f32r, name="xt")
                        nc.sync.dma_start(xts[m + 1][:], xs_d.ap()[:, m + 1])
                    xt = xts.pop(m)
                    ps = [
                        psum.tile([P, 512], f32, name="ps") for _ in range(N_TILES)
                    ]
                    for ko in range(KO):
                        for n in range(N_TILES):
                            nc.tensor.matmul(
                                ps[n][:],
                                xt[:, ko],
                                wts[ko][:, n * 512 : (n + 1) * 512],
                                start=(ko == 0),
                                stop=(ko == KO - 1),
                            )
                    for n in range(N_TILES):
                        epilogue(m, n, ps[n], bb_t)

            if loop_reps is None:
                body()
            else:
                # straight-line replication with all-engine barriers between
                # reps: timing diff (R_hi - R_lo) isolates one cold run
                for r in range(loop_reps):
                    if r:
                        tc.strict_bb_all_engine_barrier()
                    body()

    nc.compile()
    return nc


def _make_runner(nc):
    """Jitted 8-core shard_map runner for a compiled Bass module."""
    import jax
    from jax.experimental.shard_map import shard_map
    from jax.sharding import Mesh, PartitionSpec
    from concourse import mybir
    from concourse.bass2jax import (
        _bass_exec_p,
        install_neuronx_cc_hook,
        partition_id_tensor,
    )

    install_neuronx_cc_hook()

    partition_name = nc.partition_id_tensor.name if nc.partition_id_tensor else None
    in_names = []
    out_names = []
    out_avals = []
    for alloc in nc.m.functions[0].allocations:
        if not isinstance(alloc, mybir.MemoryLocationSet):
            continue
        name = alloc.memorylocations[0].name
        if alloc.kind == "ExternalInput":
            if name != partition_name:
                in_names.append(name)
        elif alloc.kind == "ExternalOutput":
            out_names.append(name)
            out_avals.append(
                jax.core.ShapedArray(
                    tuple(alloc.tensor_shape), mybir.dt.np(alloc.dtype)
                )
            )
    n_params = len(in_names)
    all_names = in_names + out_names
    if partition_name is not None:
        all_names = all_names + [partition_name]

    def _body(*args):
        operands = list(args)
        if partition_name is not None:
            operands.append(partition_id_tensor())
        outs = _bass_exec_p.bind(
            *operands,
            out_avals=tuple(out_avals),
            in_names=tuple(all_names),
            out_names=tuple(out_names),
            lowering_input_output_aliases=(),
            sim_require_finite=True,
            sim_require_nnan=True,
            nc=nc,
        )
        return tuple(outs)

    devices = jax.devices()[:N_CORES]
    mesh = Mesh(np.asarray(devices), ("core",))
    n_outs = len(out_names)
    fn = jax.jit(
        shard_map(
            _body,
            mesh=mesh,
            in_specs=(PartitionSpec("core"),) * (n_params + n_outs),
            out_specs=(PartitionSpec("core"),) * n_outs,
            check_rep=False,
        ),
        keep_unused=True,
    )
    return fn, tuple(in_names), out_avals


def _get_runner():
    if "runner" not in _STATE:
        _STATE["runner"] = _make_runner(_build_bass())
    return _STATE["runner"]


def _prepare_inputs(x, weight, bias):
    """Round + shard + swizzle. Returns dict name -> concatenated (8*dim0)
    numpy array."""
    xr = _round_fp32r(x)
    wr = _round_fp32r(weight)
    bias = np.ascontiguousarray(bias, dtype=np.float32)

    xs_l, ws_l, bb_l = [], [], []
    for c in range(N_CORES):
        bi, oj = divmod(c, P_O)
        xc = xr[bi * MB : (bi + 1) * MB, :]
        # [p, m, ko, b] = xc[m*128+b, ko*128+p]
        xs_l.append(
            np.ascontiguousarray(xc.reshape(M_TILES, P, KO, P).transpose(3, 0, 2, 1))
        )
        wc = wr[oj * NB : (oj + 1) * NB, :]
        # [p, ko, n] = wc[n, ko*128+p]
        ws_l.append(np.ascontiguousarray(wc.reshape(NB, KO, P).transpose(2, 1, 0)))
        bb_l.append(
            np.ascontiguousarray(np.broadcast_to(bias[oj * NB : (oj + 1) * NB], (P, NB)))
        )
    return {
        "xs": np.concatenate(xs_l, axis=0),
        "ws": np.concatenate(ws_l, axis=0),
        "bb": np.concatenate(bb_l, axis=0),
    }


def _assemble(out_concat: np.ndarray) -> np.ndarray:
    """[8*2048, 1024] per-core stack -> full [4096, 4096]."""
    y = np.empty((B, OUT_F), np.float32)
    per = out_concat.reshape(N_CORES, MB, NB)
    for c in range(N_CORES):
        bi, oj = divmod(c, P_O)
        y[bi * MB : (bi + 1) * MB, oj * NB : (oj + 1) * NB] = per[c]
    return y


def kernel(x: np.ndarray, weight: np.ndarray, bias: np.ndarray) -> np.ndarray:
    fn, param_names, out_avals = _get_runner()
    ins = _prepare_inputs(np.asarray(x), np.asarray(weight), np.asarray(bias))
    args = [ins[n] for n in param_names]
    zeros = [
        np.zeros((N_CORES * a.shape[0], *a.shape[1:]), a.dtype) for a in out_avals
    ]
    outs = fn(*args, *zeros)
    return _assemble(np.asarray(outs[0]))


# revision 16
# speedup vs baseline: 29.3854x; 2.9043x over previous
"""Trainium2 Bass kernel for nn_ModelNew_3556232921881 (dense_mlp).

Computes, for x[4096,4096], weight[4096,4096], bias[4096]:
    y = x @ weight.T + bias
    per-256-column-block mean subtraction (divided by out_features)
    tanh-approx GELU with clamped tanh

Sharding: 2 batch shards x 4 out-feature shards across 8 NeuronCores.
Per core: M=2048, N=1024, K=4096 GEMM (fp32r full-rate matmul) with a
fused epilogue (bias add -> block reduce -> Gelu_apprx_tanh with the
negated block mean as per-partition activation bias).

Host side pre-rounds x/weight to fp32r (fp32 with 11-bit mantissa, RNE)
and swizzles them into the exact SBUF layouts so the device does zero
transposes or dtype conversions. The W shard (16MB) is SBUF-resident;
x streams per 128-row tile. The first 4 m-tiles run k-synchronously
with the W DMA stream so the PE never waits for the W preload.
"""

import numpy as np
from contextlib import ExitStack

B, IN_F, OUT_F = 4096, 4096, 4096
P_B, P_O = 2, 4          # batch shards x out-feature shards
MB = B // P_B            # 2048 rows per core
NB = OUT_F // P_O        # 1024 out cols per core
K = IN_F
P = 128
M_TILES = MB // P        # 16
KO = K // P              # 32
N_TILES = NB // 512      # 2
N_CORES = 8
WARM_G = 4               # m-tiles processed k-synchronously with W stream

_STATE: dict = {}


def _round_fp32r(a: np.ndarray) -> np.ndarray:
    """Round fp32 to fp32r (11-bit mantissa, RNE) — matches walrus
    fp32_to_fp32r."""
    b = np.ascontiguousarray(a, dtype=np.float32).view(np.uint32)
    lsb = (b >> np.uint32(12)) & np.uint32(1)
    return ((b + np.uint32(0x7FF) + lsb) & np.uint32(0xFFFFF000)).view(np.float32)


def _build_bass(loop_reps=None, warm_group=WARM_G):
    import concourse.bass as bass  # noqa: F401
    import concourse.tile as tile
    from concourse import bacc, mybir

    f32 = mybir.dt.float32
    f32r = mybir.dt.float32r
    AF = mybir.ActivationFunctionType

    nc = bacc.Bacc("TRN2", target_bir_lowering=False, debug=False)

    # element [p, m, ko, b] = xr[m*128+b, ko*128+p]  (per-core batch shard)
    xs_d = nc.dram_tensor("xs", [P, M_TILES, KO, P], f32r, kind="ExternalInput")
    # element [p, ko, n] = w[n, ko*128+p]            (per-core outf shard)
    ws_d = nc.dram_tensor("ws", [P, KO, NB], f32r, kind="ExternalInput")
    bb_d = nc.dram_tensor("bb", [P, NB], f32, kind="ExternalInput")
    out_d = nc.dram_tensor("out", [MB, NB], f32, kind="ExternalOutput")

    with tile.TileContext(nc) as tc:
        with ExitStack() as ctx:
            wpool = ctx.enter_context(tc.tile_pool(name="w", bufs=1))
            xpool = ctx.enter_context(tc.tile_pool(name="x", bufs=max(warm_group, 2)))
            ypool = ctx.enter_context(tc.tile_pool(name="y", bufs=2))
            gpool = ctx.enter_context(tc.tile_pool(name="g", bufs=2))
            spool = ctx.enter_context(tc.tile_pool(name="s", bufs=3))
            psum = ctx.enter_context(tc.tile_pool(name="ps", bufs=8, space="PSUM"))

            def epilogue(m, n, ps_t, bb_t):
                nsl = slice(n * 512, (n + 1) * 512)
                y1 = ypool.tile([P, 512], f32, name="y1")
                nc.vector.tensor_add(y1[:], ps_t[:], bb_t[:, nsl])
                s = spool.tile([P, 2], f32, name="s")
                nc.vector.reduce_sum(
                    s[:],
                    y1[:].rearrange("p (b f) -> p b f", f=256),
                    axis=mybir.AxisListType.X,
                )
                nm = spool.tile([P, 2], f32, name="nm")
                nc.vector.tensor_scalar_mul(nm[:], s[:], -1.0 / OUT_F)
                g = gpool.tile([P, 512], f32, name="g")
                for h in range(2):
                    nc.scalar.activation(
                        g[:, h * 256 : (h + 1) * 256],
                        y1[:, h * 256 : (h + 1) * 256],
                        AF.Gelu_apprx_tanh,
                        bias=nm[:, h : h + 1],
                    )
                nc.sync.dma_start(out_d.ap()[m * P : (m + 1) * P, nsl], g[:])

            KH = KO // 2  # 16 ko per x half-tile
            WSLAB = 4     # ko per W slab DMA (2MB transfers)

            def load_x(m):
                """Two half-tiles per m (1MB DMAs, finer PE wake-up)."""
                xa = xpool.tile([P, KH, P], f32r, name="xta")
                nc.sync.dma_start(xa[:], xs_d.ap()[:, m, 0:KH])
                xb = xpool.tile([P, KH, P], f32r, name="xtb")
                nc.sync.dma_start(xb[:], xs_d.ap()[:, m, KH:KO])
                return (xa, xb)

            def x_slice(pair, ko):
                return pair[ko // KH][:, ko % KH]

            def body():
                G = warm_group
                assert G == 4, "phase-0 DMA wave is hardcoded for warm_group=4"
                # -- phase 0: DMA wave order matches the phase-1 ko-wavefront
                # consumption order. Small leading W slabs let the PE start
                # ~4us earlier; the rest of W interleaves with x halves.
                xts = {}
                slab_kos = [1, 1, 2] + [WSLAB] * ((KO - 4) // WSLAB)  # ko per slab
                assert sum(slab_kos) == KO
                slab_start = [sum(slab_kos[:i]) for i in range(len(slab_kos))]
                ko_to_slab = {}
                for i, (st, ln) in enumerate(zip(slab_start, slab_kos)):
                    for j in range(ln):
                        ko_to_slab[st + j] = (i, j)
                wts = [None] * len(slab_kos)

                def load_slab(sl):
                    st, ln = slab_start[sl], slab_kos[sl]
                    wt = wpool.tile([P, ln, NB], f32r, name=f"wt{sl}")
                    nc.sync.dma_start(wt[:], ws_d.ap()[:, st : st + ln])
                    wts[sl] = wt

                xas, xbs = [], []

                def load_xa(m):
                    xa = xpool.tile([P, KH, P], f32r, name="xta")
                    nc.sync.dma_start(xa[:], xs_d.ap()[:, m, 0:KH])
                    xas.append(xa)

                def load_xb(m):
                    xb = xpool.tile([P, KH, P], f32r, name="xtb")
                    nc.sync.dma_start(xb[:], xs_d.ap()[:, m, KH:KO])
                    xbs.append(xb)

                load_slab(0)
                load_xa(0)
                load_slab(1)
                load_xa(1)
                load_slab(2)
                load_xa(2)
                load_slab(3)
                load_xa(3)
                load_slab(4)
                load_slab(5)
                load_xb(0)
                load_xb(1)
                load_slab(6)
                load_xb(2)
                load_xb(3)
                for sl in range(7, len(slab_kos)):
                    load_slab(sl)
                for m in range(G):
                    xts[m] = (xas[m], xbs[m])

                bb_t = wpool.tile([P, NB], f32, name="bb")
                nc.sync.dma_start(bb_t[:], bb_d.ap())

                def wt_slice(ko, n):
                    sl, j = ko_to_slab[ko]
                    return wts[sl][:, j, n * 512 : (n + 1) * 512]

                # -- phase 1: warm group, k-synchronous with W arrival
                if G:
                    ps1 = {
                        (m, n): psum.tile([P, 512], f32, name="ps")
                        for m in range(G)
                        for n in range(N_TILES)
                    }
                    # diagonal wavefront: ko-blocks aligned to W slabs, m
                    # inner — each DMA arrival unlocks one block
                    for st, ln in zip(slab_start, slab_kos):
                        for m in range(G):
                            for ko in range(st, st + ln):
                                for n in range(N_TILES):
                                    nc.tensor.matmul(
                                        ps1[m, n][:],
                                        x_slice(xts[m], ko),
                                        wt_slice(ko, n),
                                        start=(ko == 0),
                                        stop=(ko == KO - 1),
                                    )
                    # prefetch next x chunk (reuses slots freed at phase-1 end)
                    if G < M_TILES:
                        xts[G] = load_x(G)
                    for m in range(G):
                        del xts[m]
                        for n in range(N_TILES):
                            epilogue(m, n, ps1[m, n], bb_t)

                # -- phase 2: remaining m-tiles, k-inner per tile
                for m in range(G, M_TILES):
                    if m + 1 < M_TILES and (m + 1) not in xts:
                        xts[m + 1] = load_x(m + 1)
                    xt = xts.pop(m)
                    ps = [
                        psum.tile([P, 512], f32, name="ps") for _ in range(N_TILES)
                    ]
                    # n-outer: ps[n] finishes its full ko sweep before ps[n+1]
                    # starts, so each epilogue hides under the next MM block
                    for n in range(N_TILES):
                        for ko in range(KO):
                            nc.tensor.matmul(
                                ps[n][:],
                                x_slice(xt, ko),
                                wt_slice(ko, n),
                                start=(ko == 0),
                                stop=(ko == KO - 1),
                            )
                        epilogue(m, n, ps[n], bb_t)

            if loop_reps is None:
                body()
            else:
                # straight-line replication with all-engine barriers between
                # reps: timing diff (R_hi - R_lo) isolates one cold run
                for r in range(loop_reps):
                    if r:
                        tc.strict_bb_all_engine_barrier()
                    body()

    nc.compile()
    return nc


def _make_runner(nc):
    """Jitted 8-core shard_map runner for a compiled Bass module."""
    import jax
    from jax.experimental.shard_map import shard_map
    from jax.sharding import Mesh, PartitionSpec
    from concourse import mybir
    from concourse.bass2jax import (
        _bass_exec_p,
        install_neuronx_cc_hook,
        partition_id_tensor,
    )

    install_neuronx_cc_hook()

    partition_name = nc.partition_id_tensor.name if nc.partition_id_tensor else None
    in_names = []
    out_names = []
    out_avals = []
    for alloc in nc.m.functions[0].allocations:
        if not isinstance(alloc, mybir.MemoryLocationSet):
            continue
        name = alloc.memorylocations[0].name
        if alloc.kind == "ExternalInput":
            if name != partition_name:
                in_names.append(name)
        elif alloc.kind == "ExternalOutput":
            out_names.append(name)
            out_avals.append(
                jax.core.ShapedArray(
                    tuple(alloc.tensor_shape), mybir.dt.np(alloc.dtype)
                )
            )
    n_params = len(in_names)
    all_names = in_names + out_names
    if partition_name is not None:
        all_names = all_names + [partition_name]

    def _body(*args):
        operands = list(args)
        if partition_name is not None:
            operands.append(partition_id_tensor())
        outs = _bass_exec_p.bind(
            *operands,
            out_avals=tuple(out_avals),
            in_names=tuple(all_names),
            out_names=tuple(out_names),
            lowering_input_output_aliases=(),
            sim_require_finite=True,
            sim_require_nnan=True,
            nc=nc,
        )
        return tuple(outs)

    devices = jax.devices()[:N_CORES]
    mesh = Mesh(np.asarray(devices), ("core",))
    n_outs = len(out_names)
    fn = jax.jit(
        shard_map(
            _body,
            mesh=mesh,
            in_specs=(PartitionSpec("core"),) * (n_params + n_outs),
            out_specs=(PartitionSpec("core"),) * n_outs,
            check_rep=False,
        ),
        keep_unused=True,
    )
    return fn, tuple(in_names), out_avals


def _get_runner():
    if "runner" not in _STATE:
        _STATE["runner"] = _make_runner(_build_bass())
    return _STATE["runner"]


def _prepare_inputs(x, weight, bias):
    """Round + shard + swizzle. Returns dict name -> concatenated (8*dim0)
    numpy array."""
    xr = _round_fp32r(x)
    wr = _round_fp32r(weight)
    bias = np.ascontiguousarray(bias, dtype=np.float32)

    xs_l, ws_l, bb_l = [], [], []
    for c in range(N_CORES):
        bi, oj = divmod(c, P_O)
        xc = xr[bi * MB : (bi + 1) * MB, :]
        # [p, m, ko, b] = xc[m*128+b, ko*128+p]
        xs_l.append(
            np.ascontiguousarray(xc.reshape(M_TILES, P, KO, P).transpose(3, 0, 2, 1))
        )
        wc = wr[oj * NB : (oj + 1) * NB, :]
        # [p, ko, n] = wc[n, ko*128+p]
        ws_l.append(np.ascontiguousarray(wc.reshape(NB, KO, P).transpose(2, 1, 0)))
        bb_l.append(
            np.ascontiguousarray(np.broadcast_to(bias[oj * NB : (oj + 1) * NB], (P, NB)))
        )
    return {
        "xs": np.concatenate(xs_l, axis=0),
        "ws": np.concatenate(ws_l, axis=0),
        "bb": np.concatenate(bb_l, axis=0),
    }


def _assemble(out_concat: np.ndarray) -> np.ndarray:
    """[8*2048, 1024] per-core stack -> full [4096, 4096]."""
    y = np.empty((B, OUT_F), np.float32)
    per = out_concat.reshape(N_CORES, MB, NB)
    for c in range(N_CORES):
        bi, oj = divmod(c, P_O)
        y[bi * MB : (bi + 1) * MB, oj * NB : (oj + 1) * NB] = per[c]
    return y


def kernel(x: np.ndarray, weight: np.ndarray, bias: np.ndarray) -> np.ndarray:
    fn, param_names, out_avals = _get_runner()
    ins = _prepare_inputs(np.asarray(x), np.asarray(weight), np.asarray(bias))
    args = [ins[n] for n in param_names]
    zeros = [
        np.zeros((N_CORES * a.shape[0], *a.shape[1:]), a.dtype) for a in out_avals
    ]
    outs = fn(*args, *zeros)
    return _assemble(np.asarray(outs[0]))


# revision 18
# speedup vs baseline: 32.1337x; 1.0935x over previous
"""Trainium2 Bass kernel for nn_ModelNew_3556232921881 (dense_mlp).

Computes, for x[4096,4096], weight[4096,4096], bias[4096]:
    y = x @ weight.T + bias
    per-256-column-block mean subtraction (divided by out_features)
    tanh-approx GELU with clamped tanh

Sharding: 2 batch shards x 4 out-feature shards across 8 NeuronCores.
Per core: M=2048, N=1024, K=4096 GEMM (fp32r full-rate matmul) with a
fused epilogue (bias add -> block reduce -> Gelu_apprx_tanh with the
negated block mean as per-partition activation bias).

Host side pre-rounds x/weight to fp32r (fp32 with 11-bit mantissa, RNE)
and swizzles them into the exact SBUF layouts so the device does zero
transposes or dtype conversions. The W shard (16MB) is SBUF-resident;
x streams per 128-row tile. The first 4 m-tiles run k-synchronously
with the W DMA stream so the PE never waits for the W preload.
"""

import numpy as np
from contextlib import ExitStack

B, IN_F, OUT_F = 4096, 4096, 4096
P_B, P_O = 2, 4          # batch shards x out-feature shards
MB = B // P_B            # 2048 rows per core
NB = OUT_F // P_O        # 1024 out cols per core
K = IN_F
P = 128
M_TILES = MB // P        # 16
KO = K // P              # 32
N_TILES = NB // 512      # 2
N_CORES = 8
WARM_G = 4               # m-tiles processed k-synchronously with W stream

_STATE: dict = {}


def _round_fp32r(a: np.ndarray) -> np.ndarray:
    """Round fp32 to fp32r (11-bit mantissa, RNE) — matches walrus
    fp32_to_fp32r."""
    b = np.ascontiguousarray(a, dtype=np.float32).view(np.uint32)
    lsb = (b >> np.uint32(12)) & np.uint32(1)
    return ((b + np.uint32(0x7FF) + lsb) & np.uint32(0xFFFFF000)).view(np.float32)


def _build_bass(loop_reps=None, warm_group=WARM_G):
    import concourse.bass as bass  # noqa: F401
    import concourse.tile as tile
    from concourse import bacc, mybir

    f32 = mybir.dt.float32
    f32r = mybir.dt.float32r
    AF = mybir.ActivationFunctionType

    nc = bacc.Bacc("TRN2", target_bir_lowering=False, debug=False)

    # element [p, m, ko, b] = xr[m*128+b, ko*128+p]  (per-core batch shard)
    xs_d = nc.dram_tensor("xs", [P, M_TILES, KO, P], f32r, kind="ExternalInput")
    # element [p, ko, n] = w[n, ko*128+p]            (per-core outf shard)
    ws_d = nc.dram_tensor("ws", [P, KO, NB], f32r, kind="ExternalInput")
    bb_d = nc.dram_tensor("bb", [P, NB], f32, kind="ExternalInput")
    out_d = nc.dram_tensor("out", [MB, NB], f32, kind="ExternalOutput")

    with tile.TileContext(nc) as tc:
        with ExitStack() as ctx:
            wpool = ctx.enter_context(tc.tile_pool(name="w", bufs=1))
            xpool = ctx.enter_context(tc.tile_pool(name="x", bufs=max(warm_group, 2)))
            ypool = ctx.enter_context(tc.tile_pool(name="y", bufs=2))
            gpool = ctx.enter_context(tc.tile_pool(name="g", bufs=2))
            spool = ctx.enter_context(tc.tile_pool(name="s", bufs=3))
            psum = ctx.enter_context(tc.tile_pool(name="ps", bufs=8, space="PSUM"))

            def epilogue(m, n, ps_t, bb_t):
                nsl = slice(n * 512, (n + 1) * 512)
                y1 = ypool.tile([P, 512], f32, name="y1")
                nc.vector.tensor_add(y1[:], ps_t[:], bb_t[:, nsl])
                s = spool.tile([P, 2], f32, name="s")
                nc.vector.reduce_sum(
                    s[:],
                    y1[:].rearrange("p (b f) -> p b f", f=256),
                    axis=mybir.AxisListType.X,
                )
                nm = spool.tile([P, 2], f32, name="nm")
                nc.vector.tensor_scalar_mul(nm[:], s[:], -1.0 / OUT_F)
                g = gpool.tile([P, 512], f32, name="g")
                for h in range(2):
                    nc.scalar.activation(
                        g[:, h * 256 : (h + 1) * 256],
                        y1[:, h * 256 : (h + 1) * 256],
                        AF.Gelu_apprx_tanh,
                        bias=nm[:, h : h + 1],
                    )
                nc.sync.dma_start(out_d.ap()[m * P : (m + 1) * P, nsl], g[:])

            KH = KO // 2  # 16 ko per x half-tile
            WSLAB = 4     # ko per W slab DMA (2MB transfers)

            def load_x(m):
                """Two half-tiles per m (1MB DMAs, finer PE wake-up)."""
                xa = xpool.tile([P, KH, P], f32r, name="xta")
                nc.sync.dma_start(xa[:], xs_d.ap()[:, m, 0:KH])
                xb = xpool.tile([P, KH, P], f32r, name="xtb")
                nc.sync.dma_start(xb[:], xs_d.ap()[:, m, KH:KO])
                return (xa, xb)

            def x_slice(pair, ko):
                return pair[ko // KH][:, ko % KH]

            def body():
                G = warm_group
                assert G == 4, "phase-0 DMA wave is hardcoded for warm_group=4"
                # -- phase 0: DMA wave order matches the phase-1 ko-wavefront
                # consumption order. Small leading W slabs let the PE start
                # ~4us earlier; the rest of W interleaves with x halves.
                xts = {}
                slab_kos = [1, 1, 2] + [WSLAB] * ((KO - 4) // WSLAB)  # ko per slab
                assert sum(slab_kos) == KO
                slab_start = [sum(slab_kos[:i]) for i in range(len(slab_kos))]
                ko_to_slab = {}
                for i, (st, ln) in enumerate(zip(slab_start, slab_kos)):
                    for j in range(ln):
                        ko_to_slab[st + j] = (i, j)
                wts = [None] * len(slab_kos)

                def load_slab(sl):
                    st, ln = slab_start[sl], slab_kos[sl]
                    wt = wpool.tile([P, ln, NB], f32r, name=f"wt{sl}")
                    nc.sync.dma_start(wt[:], ws_d.ap()[:, st : st + ln])
                    wts[sl] = wt

                xas, xbs = [], []

                def load_xa(m):
                    xa = xpool.tile([P, KH, P], f32r, name="xta")
                    nc.sync.dma_start(xa[:], xs_d.ap()[:, m, 0:KH])
                    xas.append(xa)

                def load_xb(m):
                    xb = xpool.tile([P, KH, P], f32r, name="xtb")
                    nc.sync.dma_start(xb[:], xs_d.ap()[:, m, KH:KO])
                    xbs.append(xb)

                load_slab(0)
                load_xa(0)
                load_slab(1)
                load_xa(1)
                load_slab(2)
                load_xa(2)
                load_slab(3)
                load_xa(3)
                load_slab(4)
                load_slab(5)
                load_xb(0)
                load_xb(1)
                load_slab(6)
                load_xb(2)
                load_xb(3)
                for sl in range(7, len(slab_kos)):
                    load_slab(sl)
                for m in range(G):
                    xts[m] = (xas[m], xbs[m])

                bb_t = wpool.tile([P, NB], f32, name="bb")
                nc.sync.dma_start(bb_t[:], bb_d.ap())

                def wt_slice(ko, n):
                    sl, j = ko_to_slab[ko]
                    return wts[sl][:, j, n * 512 : (n + 1) * 512]

                # -- phase 1: warm group, k-synchronous with W arrival
                if G:
                    ps1 = {
                        (m, n): psum.tile([P, 512], f32, name="ps")
                        for m in range(G)
                        for n in range(N_TILES)
                    }
                    # diagonal wavefront: ko-blocks aligned to W slabs, m
                    # inner — each DMA arrival unlocks one block
                    for st, ln in zip(slab_start, slab_kos):
                        for m in range(G):
                            for ko in range(st, st + ln):
                                for n in range(N_TILES):
                                    nc.tensor.matmul(
                                        ps1[m, n][:],
                                        x_slice(xts[m], ko),
                                        wt_slice(ko, n),
                                        start=(ko == 0),
                                        stop=(ko == KO - 1),
                                    )
                    # prefetch next x chunk (reuses slots freed at phase-1 end)
                    if G < M_TILES:
                        xts[G] = load_x(G)
                    for m in range(G):
                        del xts[m]
                        for n in range(N_TILES):
                            epilogue(m, n, ps1[m, n], bb_t)

                # -- phase 2: remaining m-tiles, k-inner per tile
                for m in range(G, M_TILES):
                    if m + 1 < M_TILES and (m + 1) not in xts:
                        xts[m + 1] = load_x(m + 1)
                    xt = xts.pop(m)
                    ps = [
                        psum.tile([P, 512], f32, name="ps") for _ in range(N_TILES)
                    ]
                    # n-outer: ps[n] finishes its full ko sweep before ps[n+1]
                    # starts, so each epilogue hides under the next MM block
                    for n in range(N_TILES):
                        for ko in range(KO):
                            nc.tensor.matmul(
                                ps[n][:],
                                x_slice(xt, ko),
                                wt_slice(ko, n),
                                start=(ko == 0),
                                stop=(ko == KO - 1),
                            )
                        epilogue(m, n, ps[n], bb_t)

            if loop_reps is None:
                body()
            else:
                # straight-line replication with all-engine barriers between
                # reps: timing diff (R_hi - R_lo) isolates one cold run
                for r in range(loop_reps):
                    if r:
                        tc.strict_bb_all_engine_barrier()
                    body()

    nc.compile()
    return nc


def _make_runner(nc):
    """Jitted 8-core shard_map runner for a compiled Bass module."""
    import jax
    from jax.experimental.shard_map import shard_map
    from jax.sharding import Mesh, PartitionSpec
    from concourse import mybir
    from concourse.bass2jax import (
        _bass_exec_p,
        install_neuronx_cc_hook,
        partition_id_tensor,
    )

    install_neuronx_cc_hook()

    partition_name = nc.partition_id_tensor.name if nc.partition_id_tensor else None
    in_names = []
    out_names = []
    out_avals = []
    for alloc in nc.m.functions[0].allocations:
        if not isinstance(alloc, mybir.MemoryLocationSet):
            continue
        name = alloc.memorylocations[0].name
        if alloc.kind == "ExternalInput":
            if name != partition_name:
                in_names.append(name)
        elif alloc.kind == "ExternalOutput":
            out_names.append(name)
            out_avals.append(
                jax.core.ShapedArray(
                    tuple(alloc.tensor_shape), mybir.dt.np(alloc.dtype)
                )
            )
    n_params = len(in_names)
    all_names = in_names + out_names
    if partition_name is not None:
        all_names = all_names + [partition_name]

    def _body(*args):
        operands = list(args)
        if partition_name is not None:
            operands.append(partition_id_tensor())
        outs = _bass_exec_p.bind(
            *operands,
            out_avals=tuple(out_avals),
            in_names=tuple(all_names),
            out_names=tuple(out_names),
            lowering_input_output_aliases=(),
            sim_require_finite=True,
            sim_require_nnan=True,
            nc=nc,
        )
        return tuple(outs)

    devices = jax.devices()[:N_CORES]
    mesh = Mesh(np.asarray(devices), ("core",))
    n_outs = len(out_names)
    fn = jax.jit(
        shard_map(
            _body,
            mesh=mesh,
            in_specs=(PartitionSpec("core"),) * (n_params + n_outs),
            out_specs=(PartitionSpec("core"),) * n_outs,
            check_rep=False,
        ),
        keep_unused=True,
    )
    return fn, tuple(in_names), out_avals


def _get_runner():
    if "runner" not in _STATE:
        _STATE["runner"] = _make_runner(_build_bass())
    return _STATE["runner"]


def _prepare_inputs(x, weight, bias):
    """Round + shard + swizzle. Returns dict name -> concatenated (8*dim0)
    numpy array."""
    xr = _round_fp32r(x)
    wr = _round_fp32r(weight)
    bias = np.ascontiguousarray(bias, dtype=np.float32)

    xs_l, ws_l, bb_l = [], [], []
    for c in range(N_CORES):
        bi, oj = divmod(c, P_O)
        xc = xr[bi * MB : (bi + 1) * MB, :]
        # [p, m, ko, b] = xc[m*128+b, ko*128+p]
        xs_l.append(
            np.ascontiguousarray(xc.reshape(M_TILES, P, KO, P).transpose(3, 0, 2, 1))
        )
        wc = wr[oj * NB : (oj + 1) * NB, :]
        # [p, ko, n] = wc[n, ko*128+p]
        ws_l.append(np.ascontiguousarray(wc.reshape(NB, KO, P).transpose(2, 1, 0)))
        bb_l.append(
            np.ascontiguousarray(np.broadcast_to(bias[oj * NB : (oj + 1) * NB], (P, NB)))
        )
    return {
        "xs": np.concatenate(xs_l, axis=0),
        "ws": np.concatenate(ws_l, axis=0),
        "bb": np.concatenate(bb_l, axis=0),
    }


def _assemble(out_concat: np.ndarray) -> np.ndarray:
    """[8*2048, 1024] per-core stack -> full [4096, 4096]."""
    y = np.empty((B, OUT_F), np.float32)
    per = out_concat.reshape(N_CORES, MB, NB)
    for c in range(N_CORES):
        bi, oj = divmod(c, P_O)
        y[bi * MB : (bi + 1) * MB, oj * NB : (oj + 1) * NB] = per[c]
    return y


def kernel(x: np.ndarray, weight: np.ndarray, bias: np.ndarray) -> np.ndarray:
    fn, param_names, out_avals = _get_runner()
    ins = _prepare_inputs(np.asarray(x), np.asarray(weight), np.asarray(bias))
    args = [ins[n] for n in param_names]
    zeros = [
        np.zeros((N_CORES * a.shape[0], *a.shape[1:]), a.dtype) for a in out_avals
    ]
    outs = fn(*args, *zeros)
    return _assemble(np.asarray(outs[0]))
